# revision 1
# baseline (speedup 1.0000x reference)
"""ColBERT MaxSim retrieval kernel for 8 Trainium2 NeuronCores.

Problem (per reference):
  Q  = l2norm(q_hidden @ W + b)                    [B, 32, 128]
  PD = l2norm((pd_hidden @ W + b) * pd_mask)       [B, 512, 128]
  ND = l2norm((nd_hidden @ W + b) * nd_mask)       [B, 512, 128]
  pos = einsum(Q, PD).max(k).sum(q);  neg likewise; out = [B, 2]

Sharding: pure data parallelism - batch dim (128) split across 8 cores
(16 batches each); W, b replicated.

Key layout decision: the PE contracts along the partition dim, so doc
hidden states must be fed as X^T [H-part, L-free]. Rather than burn PE
cycles transposing on-chip (the v0 bottleneck: 792 PE transposes +
PSUM->SBUF copies), the host pre-transposes each 512-token tile into
[128, 6, 512] bf16 blocks laid out so every per-partition DMA read is
one contiguous 6KB stream.

Normalization trick: never materialize normalized PD. With
  S_raw[q,k] = (Qn @ (Xd W + b)^T)[q,k]
  cs[k] = mask[k] * rsqrt(||Xd_k W + b||^2)
scores are S_raw * cs (masked columns exactly 0, matching the reference
where masked tokens are zero vectors), so pos = sum_q max_k (S_raw*cs).

Per-tile PE work is just: 6-chain projection matmul, one row-sum matmul
(norms, packed 4 tiles/psum-tile via tile_position), one MaxSim matmul.
Per group of 4 tiles: a single K=4 broadcast matmul expands the 4 norm
rows to the 128 score partitions.
"""

import os
import sys

import numpy as np

for _p in ("/opt/trn_rl_repo",):
    if _p not in sys.path and os.path.isdir(_p):
        sys.path.insert(0, _p)

import ml_dtypes  # noqa: E402

import concourse.bacc as bacc  # noqa: E402
import concourse.tile as tile  # noqa: E402
from concourse import mybir  # noqa: E402
from concourse.bass_utils import run_bass_kernel_spmd  # noqa: E402

# Problem shape (hardcoded per contract)
B, LQ, LD, H, D = 128, 32, 512, 768, 128
NCORES = 8
BC = B // NCORES          # 16 batches per core
KT = H // 128             # 6 contraction chunks

F32 = mybir.dt.float32
BF16 = mybir.dt.bfloat16
AF = mybir.ActivationFunctionType
ALU = mybir.AluOpType

BF16_NP = ml_dtypes.bfloat16


def build_kernel():
    nc = bacc.Bacc()

    # Pre-transposed per-tile layouts: [tile, 128 (h%128), KT*512 (h//128, l)]
    qt_d = nc.dram_tensor("qt", [128, KT * LD], BF16, kind="ExternalInput")
    pdt_d = nc.dram_tensor("pdt", [BC, 128, KT * LD], BF16, kind="ExternalInput")
    ndt_d = nc.dram_tensor("ndt", [BC, 128, KT * LD], BF16, kind="ExternalInput")
    w_d = nc.dram_tensor("W", [128, KT, 128], BF16, kind="ExternalInput")
    b_d = nc.dram_tensor("b", [D, 1], F32, kind="ExternalInput")
    # masks as [j, u, l]: BIG * (1 - mask[4u+j, l]) — added to squared norms
    # so masked tokens get inverse-norm ~1e-9 (scores ~1e-18, below tol)
    mp_d = nc.dram_tensor("mp", [4, 4, LD], BF16, kind="ExternalInput")
    mn_d = nc.dram_tensor("mn", [4, 4, LD], BF16, kind="ExternalInput")
    i4_d = nc.dram_tensor("i4", [4, 4], BF16, kind="ExternalInput")
    blk4_d = nc.dram_tensor("blk4", [4, 128], BF16, kind="ExternalInput")
    e4_d = nc.dram_tensor("e4", [128, 4], BF16, kind="ExternalInput")
    # ej4[p, j, c] = (c == j): routes a full-column reduction to out row j
    ej4_d = nc.dram_tensor("ej4", [128, 4, 4], BF16, kind="ExternalInput")
    out_d = nc.dram_tensor("out", [BC, 2], F32, kind="ExternalOutput")

    with tile.TileContext(nc) as tc:
        with (
            tc.tile_pool(name="const", bufs=1) as const,
            tc.tile_pool(name="xin", bufs=4) as xin,
            tc.tile_pool(name="ptb", bufs=4) as ptbp,
            tc.tile_pool(name="sq", bufs=5) as sqp,
            tc.tile_pool(name="small", bufs=4) as smallp,
            tc.tile_pool(name="csr", bufs=2) as csrp,
            tc.tile_pool(name="persist", bufs=1) as persist,
            tc.tile_pool(name="ptps", bufs=3, space="PSUM") as ptpsp,
            tc.tile_pool(name="ssps", bufs=2, space="PSUM") as sspsp,
            tc.tile_pool(name="s4ps", bufs=2, space="PSUM") as s4psp,
            tc.tile_pool(name="bcps", bufs=1, space="PSUM") as bcpsp,
        ):
            # ---- constants ----
            w_sb = const.tile([128, KT, 128], BF16)
            nc.sync.dma_start(out=w_sb, in_=w_d[:, :, :])
            bias_sb = const.tile([128, 1], F32)
            nc.sync.dma_start(out=bias_sb, in_=b_d[:, :])
            mp_sb = const.tile([4, 4, LD], BF16)
            nc.sync.dma_start(out=mp_sb, in_=mp_d[:, :, :])
            mn_sb = const.tile([4, 4, LD], BF16)
            nc.sync.dma_start(out=mn_sb, in_=mn_d[:, :, :])
            blk4_sb = const.tile([4, 128], BF16)
            nc.sync.dma_start(out=blk4_sb, in_=blk4_d[:, :])
            e4_sb = const.tile([128, 4], BF16)
            nc.sync.dma_start(out=e4_sb, in_=e4_d[:, :])
            ej4_sb = const.tile([128, 4, 4], BF16)
            nc.sync.dma_start(out=ej4_sb, in_=ej4_d[:, :, :])
            i4_sb = const.tile([4, 4], BF16)
            nc.sync.dma_start(out=i4_sb, in_=i4_d[:, :])
            ones_col = const.tile([128, 1], BF16)
            nc.vector.memset(ones_col, 1.0)
            ones_row = const.tile([1, 128], BF16)
            nc.vector.memset(ones_row, 1.0)

            # warm the scalar activation tables while DMAs run
            warm_sb = const.tile([1, 2], BF16)
            nc.scalar.activation(warm_sb, ones_row[0:1, 0:2], AF.Square)
            nc.scalar.activation(warm_sb, ones_row[0:1, 0:2], AF.Abs_reciprocal_sqrt)

            # warm the PE clock: ~4.5us of dummy matmuls on memset data,
            # starting right after engine init so HAM reaches full rate
            # before the first real projection chain (~13us)
            junk_w = const.tile([128, 128], BF16)
            nc.vector.memset(junk_w, 0.0)
            junk_x = const.tile([128, LD], BF16)
            nc.vector.memset(junk_x, 0.0)
            warm_ps = bcpsp.tile([128, LD], F32, tag="bc")
            for i in range(16):
                nc.tensor.matmul(
                    warm_ps, junk_w, junk_x, start=(i == 0), stop=(i == 15)
                )

            rm_sb = persist.tile([128, 8], BF16)
            qtn_sb = persist.tile([128, BC * LQ], BF16)

            def project(xt_sb):
                """6-chain matmul: xt [128, KT, 512] -> P^T psum [128, 512]."""
                pt_ps = ptpsp.tile([128, LD], F32, tag="pt")
                for k in range(KT):
                    nc.tensor.matmul(
                        pt_ps,
                        w_sb[:, k, :],
                        xt_sb[:, k, :],
                        start=(k == 0),
                        stop=(k == KT - 1),
                    )
                return pt_ps

            # ---- query stage: all 16 batches (512 query tokens) at once ----
            qx_sb = xin.tile([128, KT, LD], BF16, tag="x")
            nc.gpsimd.dma_start(
                out=qx_sb, in_=qt_d[:, :].rearrange("p (k l) -> p k l", k=KT)
            )
            qpt_ps = project(qx_sb)
            qsq_sb = sqp.tile([128, LD], BF16, tag="sq")
            nc.scalar.activation(qsq_sb, qpt_ps, AF.Square, bias=bias_sb)
            qss_ps = sspsp.tile([4, LD], F32, tag="ss")
            nc.tensor.matmul(
                qss_ps[0:1, :], ones_col, qsq_sb, start=True, stop=True
            )
            qinv_sb = smallp.tile([1, LD], BF16, tag="inv")
            nc.scalar.activation(qinv_sb, qss_ps[0:1, :], AF.Abs_reciprocal_sqrt)
            qbc_ps = bcpsp.tile([128, LD], F32, tag="bc")
            nc.tensor.matmul(qbc_ps, ones_row, qinv_sb, start=True, stop=True)
            qtb_sb = ptbp.tile([128, LD], BF16, tag="ptb")
            nc.vector.tensor_scalar_add(qtb_sb, qpt_ps, bias_sb)
            nc.vector.tensor_mul(qtn_sb, qtb_sb, qbc_ps)

            # ---- doc loop: 4 groups x {pd, nd} x 4 tiles ----
            # Group post-processing is deferred ~2 tiles into the next group
            # so its scalar/vector work doesn't contend with the next tiles'
            # bias-add/square at the group seam.
            def emit_post(u, ti, ss_ps, s4_ps, split=False):
                c = 2 * u + ti
                csrm_sb = csrp.tile([4, LD], BF16, tag="csrm")
                nc.scalar.activation(csrm_sb, ss_ps, AF.Abs_reciprocal_sqrt)
                cs_ps = bcpsp.tile([128, LD], F32, tag="bc")
                csb_sb = ptbp.tile([128, LD], BF16, tag="csb")
                scr_sb = sqp.tile([128, LD], BF16, tag="scr")
                halves = 2 if split else 1
                for h in range(halves):
                    pr = slice(128 // halves * h, 128 // halves * (h + 1))
                    rr = slice(4 // halves * h, 4 // halves * (h + 1))
                    nc.tensor.matmul(
                        cs_ps[pr, :],
                        blk4_sb[rr, pr],
                        csrm_sb[rr, :],
                        start=True,
                        stop=True,
                        tile_position=(0, 128 // halves * h),
                    )
                    nc.scalar.copy(csb_sb[pr, :], cs_ps[pr, :])
                    nc.vector.tensor_mul(scr_sb[pr, :], s4_ps[pr, :], csb_sb[pr, :])
                    nc.vector.tensor_reduce(
                        rm_sb[pr, c : c + 1],
                        scr_sb[pr, :],
                        axis=mybir.AxisListType.X,
                        op=ALU.max,
                    )

            groups = [
                (u, ti, xdram, m_sb)
                for u in range(4)
                for ti, (xdram, m_sb) in enumerate(((pdt_d, mp_sb), (ndt_d, mn_sb)))
            ]
            def emit_tile(ss_ps, s4_ps, j, b, sq_sb, ptb_sb):
                # per-tile PE post ops, emitted one tile late so the PE has
                # the next projection chain to chew on while vector/scalar
                # produce ptb/sq
                nc.tensor.matmul(
                    s4_ps[32 * j : 32 * (j + 1), :],
                    qtn_sb[:, b * LQ : (b + 1) * LQ],
                    ptb_sb,
                    start=True,
                    stop=True,
                    tile_position=(0, 32 * j),
                )
                nc.tensor.matmul(
                    ss_ps,
                    ej4_sb[:, j, :],
                    sq_sb,
                    start=(j == 0),
                    stop=False,
                )

            def emit_group_close(u, ti, ss_ps, s4_ps, m_sb):
                nc.tensor.matmul(
                    ss_ps, i4_sb, m_sb[:, u, :], start=False, stop=True
                )
                emit_post(u, ti, ss_ps, s4_ps)

            from collections import deque

            # staggered skew: entries carry a due-chain count; maxsim rides
            # 2 chains late, the ss matmul 3, so a full projection chain sits
            # between each tile's two small post-MMs (avoids back-to-back
            # LDWEIGHTS serialization)
            pend = deque()  # (due, kind, args)
            chains = 0

            def flush_after_chain():
                while pend and pend[0][0] <= chains:
                    _, kind, args = pend.popleft()
                    if kind == "m":
                        nc.tensor.matmul(
                            args[0][32 * args[1] : 32 * (args[1] + 1), :],
                            qtn_sb[:, args[2] * LQ : (args[2] + 1) * LQ],
                            args[3],
                            start=True,
                            stop=True,
                            tile_position=(0, 32 * args[1]),
                        )
                    elif kind == "s":
                        nc.tensor.matmul(
                            args[0],
                            ej4_sb[:, args[1], :],
                            args[2],
                            start=(args[1] == 0),
                            stop=False,
                        )
                    else:
                        emit_group_close(*args)

            for u, ti, xdram, m_sb in groups:
                ss_ps = sspsp.tile([4, LD], F32, tag="ss")
                s4_ps = s4psp.tile([128, LD], F32, tag="s4")
                for j in range(4):
                    b = 4 * u + j
                    first_group = (u == 0 and ti == 0)
                    if first_group and j < 2:
                        # singles up front: PE warms up without a load bubble
                        xt_sb = xin.tile([128, KT, LD], BF16, tag="x")
                        nc.gpsimd.dma_start(
                            out=xt_sb,
                            in_=xdram[b, :, :].rearrange("p (k l) -> p k l", k=KT),
                        )
                    else:
                        # pairs: amortize the ~630ns issue gap, shortening the
                        # serial load train that paces the whole kernel
                        if j % 2 == 0:
                            xt2_sb = xin.tile([128, 2, KT, LD], BF16, tag="x2")
                            nc.gpsimd.dma_start(
                                out=xt2_sb,
                                in_=xdram[b : b + 2, :, :].rearrange(
                                    "b p (k l) -> p b k l", k=KT
                                ),
                            )
                        xt_sb = xt2_sb[:, j % 2, :, :]
                    pt_ps = project(xt_sb)
                    chains += 1
                    flush_after_chain()
                    ptb_sb = ptbp.tile([128, LD], BF16, tag="ptb")
                    nc.vector.tensor_scalar_add(ptb_sb, pt_ps, bias_sb)
                    sq_sb = sqp.tile([128, LD], BF16, tag="sq")
                    nc.scalar.activation(sq_sb, pt_ps, AF.Square, bias=bias_sb)
                    pend.append((chains + 2, "m", (s4_ps, j, b, ptb_sb)))
                    pend.append((chains + 3, "s", (ss_ps, j, sq_sb)))
                pend.append((chains + 3, "g", (u, ti, ss_ps, s4_ps, m_sb)))
            while pend:
                _, kind, args = pend.popleft()
                if kind == "m":
                    nc.tensor.matmul(
                        args[0][32 * args[1] : 32 * (args[1] + 1), :],
                        qtn_sb[:, args[2] * LQ : (args[2] + 1) * LQ],
                        args[3],
                        start=True,
                        stop=True,
                        tile_position=(0, 32 * args[1]),
                    )
                elif kind == "s":
                    nc.tensor.matmul(
                        args[0],
                        ej4_sb[:, args[1], :],
                        args[2],
                        start=(args[1] == 0),
                        stop=False,
                    )
                else:
                    emit_group_close(*args)

            # ---- final reduction over queries + output ----
            o44_ps = bcpsp.tile([4, 8], F32, tag="bc")
            nc.tensor.matmul(o44_ps, e4_sb, rm_sb, start=True, stop=True)
            o44_sb = smallp.tile([4, 8], F32, tag="o44sb")
            nc.scalar.copy(o44_sb, o44_ps)
            nc.sync.dma_start(
                out=out_d[:, :].rearrange("(u g) t -> g u t", g=4),
                in_=o44_sb.rearrange("g (u t) -> g u t", t=2),
            )

    nc.compile()
    return nc


_NC_CACHE = None


def _get_nc():
    global _NC_CACHE
    if _NC_CACHE is None:
        _NC_CACHE = build_kernel()
    return _NC_CACHE


def _tileize(x):
    """[rows, H] fp32 -> [rows//512, 128, KT*512] bf16, pre-transposed."""
    nt = x.shape[0] // LD
    xb = x.astype(BF16_NP).reshape(nt, LD, KT, 128).transpose(0, 3, 2, 1)
    return np.ascontiguousarray(xb).reshape(nt, 128, KT * LD)


def _in_maps(inputs):
    q = np.asarray(inputs["q_hidden"], dtype=np.float32)
    pd = np.asarray(inputs["pd_hidden"], dtype=np.float32)
    nd = np.asarray(inputs["nd_hidden"], dtype=np.float32)
    W = np.asarray(inputs["W"], dtype=np.float32)
    b = np.ascontiguousarray(
        np.asarray(inputs["b"], dtype=np.float32).reshape(D, 1)
    )
    w_t = np.ascontiguousarray(
        W.astype(BF16_NP).reshape(KT, 128, D).transpose(1, 0, 2)
    )
    # masks [B, LD] -> per-core [4(j), 4(u), LD] bf16 = BIG * (1 - mask)
    MASK_BIG = 1.0e18
    mp = (
        (1.0 - np.asarray(inputs["pd_mask"], dtype=np.float32)) * MASK_BIG
    ).astype(BF16_NP)
    mn = (
        (1.0 - np.asarray(inputs["nd_mask"], dtype=np.float32)) * MASK_BIG
    ).astype(BF16_NP)
    i4 = np.eye(4, dtype=BF16_NP)
    blk4 = np.zeros((4, 128), dtype=BF16_NP)
    for j in range(4):
        blk4[j, 32 * j : 32 * (j + 1)] = 1
    e4 = np.zeros((128, 4), dtype=BF16_NP)
    for g in range(4):
        e4[32 * g : 32 * (g + 1), g] = 1
    ej4 = np.zeros((128, 4, 4), dtype=BF16_NP)
    for j in range(4):
        ej4[:, j, j] = 1
    maps = []
    for c in range(NCORES):
        sl = slice(c * BC, (c + 1) * BC)
        maps.append(
            {
                "qt": _tileize(q[sl].reshape(BC * LQ, H)).reshape(128, KT * LD),
                "pdt": _tileize(pd[sl].reshape(BC * LD, H)),
                "ndt": _tileize(nd[sl].reshape(BC * LD, H)),
                "W": w_t,
                "b": b,
                "mp": np.ascontiguousarray(
                    mp[sl].reshape(4, 4, LD).transpose(1, 0, 2)
                ),
                "mn": np.ascontiguousarray(
                    mn[sl].reshape(4, 4, LD).transpose(1, 0, 2)
                ),
                "blk4": blk4,
                "e4": e4,
                "ej4": ej4,
                "i4": i4,
            }
        )
    return maps


def run(inputs, **kw):
    """Run on 8 cores; returns (out [128,2] fp32, BassKernelResults)."""
    nc = _get_nc()
    res = run_bass_kernel_spmd(nc, _in_maps(inputs), list(range(NCORES)), **kw)
    out = np.concatenate(
        [np.asarray(res.results[c]["out"], dtype=np.float32) for c in range(NCORES)],
        axis=0,
    )
    return out, res


def kernel(**inputs) -> np.ndarray:
    out, _ = run(inputs)
    return out



# revision 5
# speedup vs baseline: 1.3158x; 1.3158x over previous
"""ColBERT MaxSim retrieval kernel for 8 Trainium2 NeuronCores — fp8 v2.

Problem (per reference):
  Q  = l2norm(q_hidden @ W + b)                    [B, 32, 128]
  PD = l2norm((pd_hidden @ W + b) * pd_mask)       [B, 512, 128]
  ND = l2norm((nd_hidden @ W + b) * nd_mask)       [B, 512, 128]
  pos = einsum(Q, PD).max(k).sum(q);  neg likewise; out = [B, 2]

Sharding: pure data parallelism - batch dim (128) split across 8 cores
(16 batches each); W, b replicated.

v2 strategy (v1/baseline was bf16, PE-bound at 104us):
  * Hidden states ship as fp8 e4m3 (halves HBM traffic: 26MB -> 13MB
    per core) laid out pre-transposed in DoubleRow pair-chunk order.
  * All heavy matmuls run fp8 DoubleRow (2 contraction elements per
    cell per cycle): the 768-deep projection is 3 MMs instead of 6,
    and MaxSim+norm passes process TWO 512-token tiles per MM by
    packing the pair (tile_a, tile_b) as the DoubleRow duo with
    block-diagonal stationary weights (zeros kill the cross terms).
  * Norms never materialize normalized embeddings: score columns are
    rescaled by cs[k] = mask[k] * rsqrt(||P_k||^2) after the MaxSim
    matmul (mask applied multiplicatively to cs - masked tokens score
    exactly 0, matching the reference's zero vectors).
  * Scales (powers of 2, exact in fp8/bf16): W' = 32W, so proj psum
    = 32P; ptb = psum + 32b (fp8, sigma~18, max<240); sq computed as
    (psum/16 + 2b)^2 = 4(P+b)^2 (fp8-safe); Qn shipped as 16*Qn fp8.
    s4 = 512*Qn.(P+b)-ish, cs folds rsqrt and the /512 descale via
    mask rows pre-scaled by 2^-8.
  * The multiply-by-cs + max-over-k fuse into one DVE
    tensor_tensor_reduce per 4-tile group.
"""

import os
import sys

import numpy as np

for _p in ("/opt/trn_rl_repo",):
    if _p not in sys.path and os.path.isdir(_p):
        sys.path.insert(0, _p)

import ml_dtypes  # noqa: E402

import concourse.bacc as bacc  # noqa: E402
import concourse.tile as tile  # noqa: E402
from concourse import mybir  # noqa: E402
from concourse.bass_utils import run_bass_kernel_spmd  # noqa: E402

# Problem shape (hardcoded per contract)
B, LQ, LD, H, D = 128, 32, 512, 768, 128
NCORES = 8
BC = B // NCORES          # 16 batches per core
NG = 4                    # batch groups of 4 per core
KC = 3                    # DoubleRow contraction chunks (768 = 3*256)

F32 = mybir.dt.float32
BF16 = mybir.dt.bfloat16
FP8 = mybir.dt.float8e4
AF = mybir.ActivationFunctionType
ALU = mybir.AluOpType
DR = mybir.MatmulPerfMode.DoubleRow

BF16_NP = ml_dtypes.bfloat16
FP8_NP = ml_dtypes.float8_e4m3fn

SW = 32.0                 # weight scale: W' = SW*W


def build_kernel():
    nc = bacc.Bacc()

    # fp8 pair-chunk layouts: per token tile, [128(p), c(3), i(2), n]
    # with hidden index h = 256c + 128i + p. Doc tiles group 4 batches:
    # per-partition flat order (b, c, i, n) -> one 12KB contiguous run.
    qt_d = nc.dram_tensor("qt", [128, KC * 2 * LD], FP8, kind="ExternalInput")
    pdq_d = nc.dram_tensor("pdq", [NG, 128, 4 * KC * 2 * LD], FP8, kind="ExternalInput")
    ndq_d = nc.dram_tensor("ndq", [NG, 128, 4 * KC * 2 * LD], FP8, kind="ExternalInput")
    w_d = nc.dram_tensor("W", [128, KC * 2 * D], FP8, kind="ExternalInput")
    b2_d = nc.dram_tensor("b2", [D, 2], F32, kind="ExternalInput")
    # masks [j(4), g(8), n] bf16 scaled by 2^-8 (cs descale folded in)
    mall_d = nc.dram_tensor("mall", [4, 8 * LD], BF16, kind="ExternalInput")
    # norm-row selectors [128, p(2), i(2), m(16)]
    nsel_d = nc.dram_tensor("nsel", [128, 2 * 2 * 16], FP8, kind="ExternalInput")
    blk4_d = nc.dram_tensor("blk4", [4, 128], BF16, kind="ExternalInput")
    e4_d = nc.dram_tensor("e4", [128, 4], BF16, kind="ExternalInput")
    out_d = nc.dram_tensor("out", [BC, 2], F32, kind="ExternalOutput")

    with tile.TileContext(nc) as tc:
        with (
            tc.tile_pool(name="const", bufs=1) as const,
            tc.tile_pool(name="xin", bufs=2) as xin,
            tc.tile_pool(name="ptb", bufs=2) as ptbp,
            tc.tile_pool(name="sq", bufs=2) as sqp,
            tc.tile_pool(name="small", bufs=2) as smallp,
            tc.tile_pool(name="csr", bufs=2) as csrp,
            tc.tile_pool(name="csb", bufs=2) as csbp,
            tc.tile_pool(name="persist", bufs=1) as persist,
            tc.tile_pool(name="ptps", bufs=2, space="PSUM") as ptpsp,
            tc.tile_pool(name="s4ps", bufs=2, space="PSUM") as s4psp,
            tc.tile_pool(name="ssps", bufs=1, space="PSUM") as sspsp,
            tc.tile_pool(name="bcps", bufs=1, space="PSUM") as bcpsp,
        ):
            # ---- input DMAs first: query, then doc group 0 as two
            # pair-halves (lower latency to first doc chain), then quads
            qx_sb = const.tile([128, KC, 2, LD], FP8)
            nc.gpsimd.dma_start(
                out=qx_sb, in_=qt_d[:, :].rearrange("p (c i l) -> p c i l", c=KC, i=2)
            )
            xg_tiles = []
            g_list = [(u, s) for u in range(NG) for s in range(2)]
            xq0_sb = xin.tile([128, 4, KC, 2, LD], FP8, tag="x")
            half = 4 * KC * 2 * LD // 2
            nc.gpsimd.dma_start(
                out=xq0_sb[:, 0:2, :, :, :],
                in_=pdq_d[0, :, 0:half].rearrange(
                    "p (b c i l) -> p b c i l", b=2, c=KC, i=2
                ),
            )
            nc.gpsimd.dma_start(
                out=xq0_sb[:, 2:4, :, :, :],
                in_=pdq_d[0, :, half:].rearrange(
                    "p (b c i l) -> p b c i l", b=2, c=KC, i=2
                ),
            )

            # ---- constants (sync queue; small) ----
            w_sb = const.tile([128, KC, 2, D], FP8)
            nc.sync.dma_start(
                out=w_sb, in_=w_d[:, :].rearrange("p (c i m) -> p c i m", c=KC, i=2)
            )
            b2_sb = const.tile([128, 2], F32)
            nc.sync.dma_start(out=b2_sb, in_=b2_d[:, :])
            mall_sb = const.tile([4, 8, LD], BF16)
            nc.sync.dma_start(
                out=mall_sb, in_=mall_d[:, :].rearrange("j (g l) -> j g l", g=8)
            )
            nsel_sb = const.tile([128, 2, 2, 16], FP8)
            nc.sync.dma_start(
                out=nsel_sb,
                in_=nsel_d[:, :].rearrange("p (q i m) -> p q i m", q=2, i=2),
            )
            blk4_sb = const.tile([4, 128], BF16)
            nc.sync.dma_start(out=blk4_sb, in_=blk4_d[:, :])
            e4_sb = const.tile([128, 4], BF16)
            nc.sync.dma_start(out=e4_sb, in_=e4_d[:, :])

            ones_col = const.tile([128, 1], BF16)
            nc.vector.memset(ones_col, 1.0)
            ones_row = const.tile([1, 128], BF16)
            nc.vector.memset(ones_row, 1.0)
            # Qn stationary pairs [128, u(4), p(2), i(2), m(128)]; zeros
            # everywhere except the block-diagonal Qn slots
            qpair_sb = persist.tile([128, NG, 2, 2, 128], FP8)
            nc.vector.memset(qpair_sb, 0.0)
            rm_sb = persist.tile([128, 8], BF16)

            # warm the scalar activation tables while DMAs run
            warm_sb = const.tile([1, 2], BF16)
            nc.scalar.activation(warm_sb, ones_row[0:1, 0:2], AF.Square)
            nc.scalar.activation(warm_sb, ones_row[0:1, 0:2], AF.Abs_reciprocal_sqrt)

            # ---- query stage: 512 query tokens in one tile ----
            qpt_ps = ptpsp.tile([128, 2, LD], F32, tag="pt")
            for c in range(KC):
                nc.tensor.matmul(
                    qpt_ps[:, 0, :],
                    w_sb[:, c, :, :],
                    qx_sb[:, c, :, :],
                    start=(c == 0),
                    stop=(c == KC - 1),
                    perf_mode=DR,
                )
            qtb_sb = const.tile([128, LD], BF16)
            nc.vector.tensor_scalar_add(qtb_sb, qpt_ps[:, 0, :], b2_sb[:, 0:1])
            qsq_sb = const.tile([128, LD], BF16)
            nc.scalar.activation(
                qsq_sb, qpt_ps[:, 0, :], AF.Square, bias=b2_sb[:, 1:2], scale=1.0 / 16
            )
            qss_ps = sspsp.tile([1, LD], F32, tag="ss")
            nc.tensor.matmul(qss_ps, ones_col, qsq_sb, start=True, stop=True)
            qinv_sb = smallp.tile([1, LD], BF16, tag="inv")
            nc.scalar.activation(qinv_sb, qss_ps, AF.Abs_reciprocal_sqrt)
            qbc_ps = bcpsp.tile([128, LD], F32, tag="bc")
            nc.tensor.matmul(qbc_ps, ones_row, qinv_sb, start=True, stop=True)
            # scatter 16*Qn into the pair-stationary slots: batch 4u+j
            # -> qpair[:, u, j//2, j%2, 64*(j//2)+32*(j%2) + 0:32]
            qtb_v = qtb_sb.rearrange("p (u j q) -> p u j q", u=4, j=4)
            qbc_v = qbc_ps.rearrange("p (u j q) -> p u j q", u=4, j=4)
            for j in range(4):
                cb = 64 * (j // 2) + 32 * (j % 2)
                nc.vector.tensor_tensor(
                    out=qpair_sb[:, :, j // 2, j % 2, cb : cb + 32],
                    in0=qtb_v[:, :, j, :],
                    in1=qbc_v[:, :, j, :],
                    op=ALU.mult,
                )

            # ---- doc loop: 8 groups (u, side) x 2 pairs x 2 tiles ----
            scrj_sb = persist.tile([128, LD], BF16)  # ttr full-out sink

            pend = []  # (due_pair, kind, args)

            def flush(k):
                while pend and pend[0][0] <= k:
                    _, kind, args = pend.pop(0)
                    if kind == "ms":
                        s4_ps, u, p, ptbp_sb = args
                        nc.tensor.matmul(
                            s4_ps,
                            qpair_sb[:, u, p, :, :],
                            ptbp_sb,
                            start=(p == 0),
                            stop=(p == 1),
                            perf_mode=DR,
                        )
                    elif kind == "ns":
                        ss_ps, p, sq_sb = args
                        nc.tensor.matmul(
                            ss_ps,
                            nsel_sb[:, p, :, :],
                            sq_sb,
                            start=(p == 0),
                            stop=(p == 1),
                            perf_mode=DR,
                        )
                    elif kind == "cs":
                        # close part 1: rsqrt + mask (ACT/DVE only)
                        g, ss_ps = args
                        csrm_sb = csrp.tile([4, LD], BF16, tag="csrm")
                        nc.scalar.activation(
                            csrm_sb, ss_ps[0:4, :], AF.Abs_reciprocal_sqrt
                        )
                        csrmm_sb = csrp.tile([4, LD], BF16, tag="csrmm")
                        nc.vector.tensor_tensor(
                            out=csrmm_sb,
                            in0=csrm_sb,
                            in1=mall_sb[:, g, :],
                            op=ALU.mult,
                        )
                        pend_close[g] = csrmm_sb
                    else:  # "bc": close part 2 (PE bcast + csb + ttr)
                        g, s4_ps = args
                        csrmm_sb = pend_close.pop(g)
                        bc_ps = bcpsp.tile([128, LD], F32, tag="bc")
                        nc.tensor.matmul(
                            bc_ps, blk4_sb, csrmm_sb, start=True, stop=True
                        )
                        csb_sb = csbp.tile([128, LD], BF16, tag="csb")
                        nc.scalar.copy(csb_sb, bc_ps)
                        nc.vector.tensor_tensor(
                            out=scrj_sb, in0=s4_ps, in1=csb_sb, op=ALU.mult
                        )
                        nc.vector.tensor_reduce(
                            rm_sb[:, g : g + 1],
                            scrj_sb,
                            axis=mybir.AxisListType.X,
                            op=ALU.max,
                        )

            pend_close = {}
            kpair = 0
            for g, (u, side) in enumerate(g_list):
                xd = pdq_d if side == 0 else ndq_d
                if g == 0:
                    xq_sb = xq0_sb
                else:
                    xq_sb = xin.tile([128, 4, KC, 2, LD], FP8, tag="x")
                    nc.gpsimd.dma_start(
                        out=xq_sb,
                        in_=xd[u, :, :].rearrange(
                            "p (b c i l) -> p b c i l", b=4, c=KC, i=2
                        ),
                    )
                ss_ps = sspsp.tile([16, LD], F32, tag="ss")
                s4_ps = s4psp.tile([128, LD], F32, tag="s4")
                for p in range(2):
                    pt_ps = ptpsp.tile([128, 2, LD], F32, tag="pt")
                    for t in range(2):
                        for c in range(KC):
                            nc.tensor.matmul(
                                pt_ps[:, t, :],
                                w_sb[:, c, :, :],
                                xq_sb[:, 2 * p + t, c, :, :],
                                start=(c == 0),
                                stop=(c == KC - 1),
                                perf_mode=DR,
                            )
                    kpair += 1
                    # pair post FIRST: keeps ptb/sq ahead of close work in
                    # the DVE/ACT queues so the deferred MMs never stall
                    ptbp_sb = ptbp.tile([128, 2, LD], FP8, tag="ptb")
                    nc.vector.tensor_scalar_add(ptbp_sb, pt_ps, b2_sb[:, 0:1])
                    sq_sb = sqp.tile([128, 2, LD], FP8, tag="sq")
                    nc.scalar.activation(
                        sq_sb, pt_ps, AF.Square, bias=b2_sb[:, 1:2], scale=1.0 / 16
                    )
                    flush(kpair)
                    pend.append((kpair + 1, "ms", (s4_ps, u, p, ptbp_sb)))
                    pend.append((kpair + 1, "ns", (ss_ps, p, sq_sb)))
                pend.append((kpair + 1, "cs", (g, ss_ps)))
                pend.append((kpair + 2, "bc", (g, s4_ps)))
            flush(10**9)

            # ---- final reduction over queries + output ----
            o44_ps = bcpsp.tile([4, 8], F32, tag="bc")
            nc.tensor.matmul(o44_ps, e4_sb, rm_sb, start=True, stop=True)
            o44_sb = smallp.tile([4, 8], F32, tag="o44sb")
            nc.scalar.copy(o44_sb, o44_ps)
            nc.sync.dma_start(
                out=out_d[:, :].rearrange("(u g) t -> g u t", g=4),
                in_=o44_sb.rearrange("g (u t) -> g u t", t=2),
            )

    nc.compile()
    return nc


_NC_CACHE = None


def _get_nc():
    global _NC_CACHE
    if _NC_CACHE is None:
        _NC_CACHE = build_kernel()
    return _NC_CACHE


def _fp8(x):
    return np.clip(x, -240.0, 240.0).astype(FP8_NP)


def _pack_docs(x):
    """[16, 512, H] fp32 -> [4(u), 128, 12288] fp8 pair-chunk quads."""
    xq = x.reshape(NG, 4, LD, KC, 2, 128).transpose(0, 5, 1, 3, 4, 2)
    return np.ascontiguousarray(_fp8(xq)).reshape(NG, 128, 4 * KC * 2 * LD)


def _in_maps(inputs):
    q = np.asarray(inputs["q_hidden"], dtype=np.float32)
    pd = np.asarray(inputs["pd_hidden"], dtype=np.float32)
    nd = np.asarray(inputs["nd_hidden"], dtype=np.float32)
    W = np.asarray(inputs["W"], dtype=np.float32)
    b = np.asarray(inputs["b"], dtype=np.float32)

    w_t = np.ascontiguousarray(
        _fp8(SW * W).reshape(KC, 2, 128, D).transpose(2, 0, 1, 3)
    ).reshape(128, KC * 2 * D)
    b2 = np.ascontiguousarray(
        np.stack([SW * b, 2.0 * b], axis=1).astype(np.float32)
    )
    mp = np.asarray(inputs["pd_mask"], dtype=np.float32) * (2.0 ** -8)
    mn = np.asarray(inputs["nd_mask"], dtype=np.float32) * (2.0 ** -8)

    nsel = np.zeros((128, 2, 2, 16), dtype=FP8_NP)
    for p in range(2):
        nsel[:, p, 0, 2 * p] = 1.0
        nsel[:, p, 1, 2 * p + 1] = 1.0
    nsel = nsel.reshape(128, 64)
    blk4 = np.zeros((4, 128), dtype=BF16_NP)
    for j in range(4):
        blk4[j, 32 * j : 32 * (j + 1)] = 1
    e4 = np.zeros((128, 4), dtype=BF16_NP)
    for gg in range(4):
        e4[32 * gg : 32 * (gg + 1), gg] = 1

    maps = []
    for cix in range(NCORES):
        sl = slice(cix * BC, (cix + 1) * BC)
        # query: tokens b-major as one 512-col tile
        qc = q[sl].reshape(BC * LQ, KC, 2, 128).transpose(3, 1, 2, 0)
        # masks -> [j(4), g(8)=2u+side, 512] bf16
        mall = np.zeros((4, 8, LD), dtype=np.float32)
        for u in range(NG):
            for j in range(4):
                mall[j, 2 * u + 0] = mp[sl][4 * u + j]
                mall[j, 2 * u + 1] = mn[sl][4 * u + j]
        maps.append(
            {
                "qt": np.ascontiguousarray(_fp8(qc)).reshape(128, KC * 2 * LD),
                "pdq": _pack_docs(pd[sl]),
                "ndq": _pack_docs(nd[sl]),
                "W": w_t,
                "b2": b2,
                "mall": mall.astype(BF16_NP).reshape(4, 8 * LD),
                "nsel": nsel,
                "blk4": blk4,
                "e4": e4,
            }
        )
    return maps


def run(inputs, **kw):
    """Run on 8 cores; returns (out [128,2] fp32, BassKernelResults)."""
    nc = _get_nc()
    res = run_bass_kernel_spmd(nc, _in_maps(inputs), list(range(NCORES)), **kw)
    out = np.concatenate(
        [np.asarray(res.results[c]["out"], dtype=np.float32) for c in range(NCORES)],
        axis=0,
    )
    return out, res


def kernel(**inputs) -> np.ndarray:
    out, _ = run(inputs)
    return out


# revision 9
# speedup vs baseline: 1.4137x; 1.0744x over previous
"""ColBERT MaxSim retrieval kernel for 8 Trainium2 NeuronCores — fp8 v2.

Problem (per reference):
  Q  = l2norm(q_hidden @ W + b)                    [B, 32, 128]
  PD = l2norm((pd_hidden @ W + b) * pd_mask)       [B, 512, 128]
  ND = l2norm((nd_hidden @ W + b) * nd_mask)       [B, 512, 128]
  pos = einsum(Q, PD).max(k).sum(q);  neg likewise; out = [B, 2]

Sharding: pure data parallelism - batch dim (128) split across 8 cores
(16 batches each); W, b replicated.

v2 strategy (v1/baseline was bf16, PE-bound at 104us):
  * Hidden states ship as fp8 e4m3 (halves HBM traffic: 26MB -> 13MB
    per core) laid out pre-transposed in DoubleRow pair-chunk order.
  * All heavy matmuls run fp8 DoubleRow (2 contraction elements per
    cell per cycle): the 768-deep projection is 3 MMs instead of 6,
    and MaxSim+norm passes process TWO 512-token tiles per MM by
    packing the pair (tile_a, tile_b) as the DoubleRow duo with
    block-diagonal stationary weights (zeros kill the cross terms).
  * Norms never materialize normalized embeddings: score columns are
    rescaled by cs[k] = mask[k] * rsqrt(||P_k||^2) after the MaxSim
    matmul (mask applied multiplicatively to cs - masked tokens score
    exactly 0, matching the reference's zero vectors).
  * Scales (powers of 2, exact in fp8/bf16): W' = 32W, so proj psum
    = 32P; ptb = psum + 32b (fp8, sigma~18, max<240); sq computed as
    (psum/16 + 2b)^2 = 4(P+b)^2 (fp8-safe); Qn shipped as 16*Qn fp8.
    s4 = 512*Qn.(P+b)-ish, cs folds rsqrt and the /512 descale via
    mask rows pre-scaled by 2^-8.
  * The multiply-by-cs + max-over-k fuse into one DVE
    tensor_tensor_reduce per 4-tile group.
"""

import os
import sys

import numpy as np

for _p in ("/opt/trn_rl_repo",):
    if _p not in sys.path and os.path.isdir(_p):
        sys.path.insert(0, _p)

import ml_dtypes  # noqa: E402

import concourse.bacc as bacc  # noqa: E402
import concourse.tile as tile  # noqa: E402
from concourse import mybir  # noqa: E402
from concourse.bass_utils import run_bass_kernel_spmd  # noqa: E402

# Problem shape (hardcoded per contract)
B, LQ, LD, H, D = 128, 32, 512, 768, 128
NCORES = 8
BC = B // NCORES          # 16 batches per core
NG = 4                    # batch groups of 4 per core
KC = 3                    # DoubleRow contraction chunks (768 = 3*256)

F32 = mybir.dt.float32
BF16 = mybir.dt.bfloat16
FP8 = mybir.dt.float8e4
AF = mybir.ActivationFunctionType
ALU = mybir.AluOpType
DR = mybir.MatmulPerfMode.DoubleRow

BF16_NP = ml_dtypes.bfloat16
FP8_NP = ml_dtypes.float8_e4m3fn

SW = 32.0                 # weight scale: W' = SW*W


def build_kernel():
    nc = bacc.Bacc()

    # fp8 pair-chunk layouts: per token tile, [128(p), c(3), i(2), n]
    # with hidden index h = 256c + 128i + p. Doc tiles group 4 batches:
    # per-partition flat order (b, c, i, n) -> one 12KB contiguous run.
    qt_d = nc.dram_tensor("qt", [128, KC * 2 * LD], FP8, kind="ExternalInput")
    pdq_d = nc.dram_tensor("pdq", [NG, 128, 4 * KC * 2 * LD], FP8, kind="ExternalInput")
    ndq_d = nc.dram_tensor("ndq", [NG, 128, 4 * KC * 2 * LD], FP8, kind="ExternalInput")
    w_d = nc.dram_tensor("W", [128, KC * 2 * D], FP8, kind="ExternalInput")
    b2_d = nc.dram_tensor("b2", [D, 2], F32, kind="ExternalInput")
    # masks [j(4), g(8), n] bf16 scaled by 2^-8 (cs descale folded in)
    mall_d = nc.dram_tensor("mall", [4, 8 * LD], BF16, kind="ExternalInput")
    # norm-row selectors [128, p(2), i(2), m(16)]
    nsel_d = nc.dram_tensor("nsel", [128, 2 * 2 * 16], FP8, kind="ExternalInput")
    blk4_d = nc.dram_tensor("blk4", [4, 128], BF16, kind="ExternalInput")
    e4_d = nc.dram_tensor("e4", [128, 4], BF16, kind="ExternalInput")
    out_d = nc.dram_tensor("out", [BC, 2], F32, kind="ExternalOutput")

    with tile.TileContext(nc) as tc:
        with (
            tc.tile_pool(name="const", bufs=1) as const,
            tc.tile_pool(name="xin", bufs=3) as xin,
            tc.tile_pool(name="ptb", bufs=3) as ptbp,
            tc.tile_pool(name="sq", bufs=3) as sqp,
            tc.tile_pool(name="small", bufs=2) as smallp,
            tc.tile_pool(name="csr", bufs=2) as csrp,
            tc.tile_pool(name="csb", bufs=2) as csbp,
            tc.tile_pool(name="persist", bufs=1) as persist,
            tc.tile_pool(name="ptps", bufs=2, space="PSUM") as ptpsp,
            tc.tile_pool(name="s4ps", bufs=2, space="PSUM") as s4psp,
            tc.tile_pool(name="ssps", bufs=1, space="PSUM") as sspsp,
            tc.tile_pool(name="bcps", bufs=1, space="PSUM") as bcpsp,
        ):
            # ---- input DMAs first: query, then doc group 0 as two
            # pair-halves (lower latency to first doc chain), then quads
            qx_sb = const.tile([128, KC, 2, LD], FP8)
            nc.gpsimd.dma_start(
                out=qx_sb, in_=qt_d[:, :].rearrange("p (c i l) -> p c i l", c=KC, i=2)
            )
            xg_tiles = []
            g_list = [(u, s) for u in range(NG) for s in range(2)]
            xq0_sb = xin.tile([128, 4, KC, 2, LD], FP8, tag="x")
            half = 4 * KC * 2 * LD // 2
            nc.gpsimd.dma_start(
                out=xq0_sb[:, 0:2, :, :, :],
                in_=pdq_d[0, :, 0:half].rearrange(
                    "p (b c i l) -> p b c i l", b=2, c=KC, i=2
                ),
            )
            nc.gpsimd.dma_start(
                out=xq0_sb[:, 2:4, :, :, :],
                in_=pdq_d[0, :, half:].rearrange(
                    "p (b c i l) -> p b c i l", b=2, c=KC, i=2
                ),
            )

            # ---- constants (sync queue; small) ----
            w_sb = const.tile([128, KC, 2, D], FP8)
            nc.sync.dma_start(
                out=w_sb, in_=w_d[:, :].rearrange("p (c i m) -> p c i m", c=KC, i=2)
            )
            b2_sb = const.tile([128, 2], F32)
            nc.sync.dma_start(out=b2_sb, in_=b2_d[:, :])
            mall_sb = const.tile([4, 8, LD], BF16)
            nc.sync.dma_start(
                out=mall_sb, in_=mall_d[:, :].rearrange("j (g l) -> j g l", g=8)
            )
            nsel_sb = const.tile([128, 2, 2, 16], FP8)
            nc.sync.dma_start(
                out=nsel_sb,
                in_=nsel_d[:, :].rearrange("p (q i m) -> p q i m", q=2, i=2),
            )
            blk4_sb = const.tile([4, 128], BF16)
            nc.sync.dma_start(out=blk4_sb, in_=blk4_d[:, :])
            e4_sb = const.tile([128, 4], BF16)
            nc.sync.dma_start(out=e4_sb, in_=e4_d[:, :])

            ones_col = const.tile([128, 1], BF16)
            nc.vector.memset(ones_col, 1.0)
            ones_row = const.tile([1, 128], BF16)
            nc.vector.memset(ones_row, 1.0)
            # Qn stationary pairs [128, u(4), p(2), i(2), m(128)]; zeros
            # everywhere except the block-diagonal Qn slots
            qpair_sb = persist.tile([128, NG, 2, 2, 128], FP8)
            nc.vector.memset(qpair_sb, 0.0)
            rm_sb = persist.tile([128, 8], BF16)

            # warm the scalar activation tables while DMAs run
            warm_sb = const.tile([1, 2], BF16)
            nc.scalar.activation(warm_sb, ones_row[0:1, 0:2], AF.Square)
            nc.scalar.activation(warm_sb, ones_row[0:1, 0:2], AF.Abs_reciprocal_sqrt)

            # warm the PE clock during the DMA-wait window: ~2.6us of
            # N=256 junk matmuls so HAM hits 2.4GHz before real work
            junk_w = const.tile([128, 128], BF16)
            nc.vector.memset(junk_w, 0.0)
            junk_x = const.tile([128, 256], BF16)
            nc.vector.memset(junk_x, 0.0)
            warm_ps = bcpsp.tile([128, 256], F32, tag="bc")
            for i in range(12):
                nc.tensor.matmul(
                    warm_ps, junk_w, junk_x, start=(i == 0), stop=(i == 11)
                )

            # ---- query stage: 512 query tokens in one tile. Only the
            # projection chains run now; the norm/scatter stages are
            # deferred into the doc loop so its chains fill the gaps.
            qpt_ps = ptpsp.tile([128, 2, LD], F32, tag="pt")
            for c in range(KC):
                nc.tensor.matmul(
                    qpt_ps[:, 0, :],
                    w_sb[:, c, :, :],
                    qx_sb[:, c, :, :],
                    start=(c == 0),
                    stop=(c == KC - 1),
                    perf_mode=DR,
                )
            qtb_sb = const.tile([128, LD], BF16)
            nc.vector.tensor_scalar_add(qtb_sb, qpt_ps[:, 0, :], b2_sb[:, 0:1])
            qsq_sb = const.tile([128, LD], BF16)
            nc.scalar.activation(
                qsq_sb, qpt_ps[:, 0, :], AF.Square, bias=b2_sb[:, 1:2], scale=1.0 / 16
            )

            def emit_qss():
                qss_ps = sspsp.tile([1, LD], F32, tag="ss")
                nc.tensor.matmul(qss_ps, ones_col, qsq_sb, start=True, stop=True)
                pend_q["ss"] = qss_ps

            def emit_qscatter():
                qss_ps = pend_q.pop("ss")
                qinv_sb = smallp.tile([1, LD], BF16, tag="inv")
                nc.scalar.activation(qinv_sb, qss_ps, AF.Abs_reciprocal_sqrt)
                qbc_ps = bcpsp.tile([128, LD], F32, tag="bc")
                nc.tensor.matmul(qbc_ps, ones_row, qinv_sb, start=True, stop=True)
                # scatter 16*Qn into the pair-stationary slots: batch 4u+j
                # -> qpair[:, u, j//2, j%2, 64*(j//2)+32*(j%2) + 0:32]
                qtb_v = qtb_sb.rearrange("p (u j q) -> p u j q", u=4, j=4)
                qbc_v = qbc_ps.rearrange("p (u j q) -> p u j q", u=4, j=4)
                for j in range(4):
                    cb = 64 * (j // 2) + 32 * (j % 2)
                    nc.vector.tensor_tensor(
                        out=qpair_sb[:, :, j // 2, j % 2, cb : cb + 32],
                        in0=qtb_v[:, :, j, :],
                        in1=qbc_v[:, :, j, :],
                        op=ALU.mult,
                    )

            # ---- doc loop: 8 groups (u, side) x 2 pairs x 2 tiles ----
            scrj_sb = persist.tile([128, LD], BF16)  # ttr full-out sink

            pend = []  # (due_pair, kind, args)

            def flush(k):
                while pend and pend[0][0] <= k:
                    _, kind, args = pend.pop(0)
                    if kind == "ms":
                        s4_ps, u, p, ptbp_sb = args
                        nc.tensor.matmul(
                            s4_ps,
                            qpair_sb[:, u, p, :, :],
                            ptbp_sb,
                            start=(p == 0),
                            stop=(p == 1),
                            perf_mode=DR,
                        )
                    elif kind == "ns":
                        ss_ps, p, sq_sb = args
                        nc.tensor.matmul(
                            ss_ps,
                            nsel_sb[:, p, :, :],
                            sq_sb,
                            start=(p == 0),
                            stop=(p == 1),
                            perf_mode=DR,
                        )
                    elif kind == "qs":
                        emit_qss()
                    elif kind == "qb":
                        emit_qscatter()
                    elif kind == "cs":
                        # close part 1: rsqrt + mask (ACT/DVE only)
                        g, ss_ps = args
                        csrm_sb = csrp.tile([4, LD], BF16, tag="csrm")
                        nc.scalar.activation(
                            csrm_sb, ss_ps[0:4, :], AF.Abs_reciprocal_sqrt
                        )
                        csrmm_sb = csrp.tile([4, LD], BF16, tag="csrmm")
                        nc.vector.tensor_tensor(
                            out=csrmm_sb,
                            in0=csrm_sb,
                            in1=mall_sb[:, g, :],
                            op=ALU.mult,
                        )
                        pend_close[g] = csrmm_sb
                    else:  # "bc": close part 2 (PE bcast + csb + ttr)
                        g, s4_ps = args
                        csrmm_sb = pend_close.pop(g)
                        bc_ps = bcpsp.tile([128, LD], F32, tag="bc")
                        nc.tensor.matmul(
                            bc_ps, blk4_sb, csrmm_sb, start=True, stop=True
                        )
                        csb_sb = csbp.tile([128, LD], BF16, tag="csb")
                        nc.scalar.copy(csb_sb, bc_ps)
                        nc.vector.tensor_tensor(
                            out=scrj_sb, in0=s4_ps, in1=csb_sb, op=ALU.mult
                        )
                        nc.vector.tensor_reduce(
                            rm_sb[:, g : g + 1],
                            scrj_sb,
                            axis=mybir.AxisListType.X,
                            op=ALU.max,
                        )

            pend_close = {}
            pend_q = {}
            pend.append((1, "qs", ()))
            pend.append((2, "qb", ()))
            kpair = 0
            for g, (u, side) in enumerate(g_list):
                xd = pdq_d if side == 0 else ndq_d
                if g == 0:
                    xq_sb = xq0_sb
                else:
                    xq_sb = xin.tile([128, 4, KC, 2, LD], FP8, tag="x")
                    nc.gpsimd.dma_start(
                        out=xq_sb,
                        in_=xd[u, :, :].rearrange(
                            "p (b c i l) -> p b c i l", b=4, c=KC, i=2
                        ),
                    )
                ss_ps = sspsp.tile([16, LD], F32, tag="ss")
                s4_ps = s4psp.tile([128, LD], F32, tag="s4")
                for p in range(2):
                    pt_ps = ptpsp.tile([128, 2, LD], F32, tag="pt")
                    # chunk-major across the pair: consecutive MMs share
                    # the same stationary W chunk
                    for c in range(KC):
                        for t in range(2):
                            nc.tensor.matmul(
                                pt_ps[:, t, :],
                                w_sb[:, c, :, :],
                                xq_sb[:, 2 * p + t, c, :, :],
                                start=(c == 0),
                                stop=(c == KC - 1),
                                perf_mode=DR,
                            )
                    kpair += 1
                    # pair post FIRST: keeps ptb/sq ahead of close work in
                    # the DVE/ACT queues so the deferred MMs never stall
                    ptbp_sb = ptbp.tile([128, 2, LD], FP8, tag="ptb")
                    nc.vector.tensor_scalar_add(ptbp_sb, pt_ps, b2_sb[:, 0:1])
                    sq_sb = sqp.tile([128, 2, LD], FP8, tag="sq")
                    nc.scalar.activation(
                        sq_sb, pt_ps, AF.Square, bias=b2_sb[:, 1:2], scale=1.0 / 16
                    )
                    flush(kpair)
                    pend.append((kpair + 2, "ms", (s4_ps, u, p, ptbp_sb)))
                    pend.append((kpair + 2, "ns", (ss_ps, p, sq_sb)))
                pend.append((kpair + 2, "cs", (g, ss_ps)))
                pend.append((kpair + 3, "bc", (g, s4_ps)))
            flush(10**9)

            # ---- final reduction over queries + output ----
            o44_ps = bcpsp.tile([4, 8], F32, tag="bc")
            nc.tensor.matmul(o44_ps, e4_sb, rm_sb, start=True, stop=True)
            o44_sb = smallp.tile([4, 8], F32, tag="o44sb")
            nc.scalar.copy(o44_sb, o44_ps)
            nc.sync.dma_start(
                out=out_d[:, :].rearrange("(u g) t -> g u t", g=4),
                in_=o44_sb.rearrange("g (u t) -> g u t", t=2),
            )

    nc.compile()
    return nc


_NC_CACHE = None


def _get_nc():
    global _NC_CACHE
    if _NC_CACHE is None:
        _NC_CACHE = build_kernel()
    return _NC_CACHE


def _fp8(x):
    return np.clip(x, -240.0, 240.0).astype(FP8_NP)


def _pack_docs(x):
    """[16, 512, H] fp32 -> [4(u), 128, 12288] fp8 pair-chunk quads."""
    xq = x.reshape(NG, 4, LD, KC, 2, 128).transpose(0, 5, 1, 3, 4, 2)
    return np.ascontiguousarray(_fp8(xq)).reshape(NG, 128, 4 * KC * 2 * LD)


def _in_maps(inputs):
    q = np.asarray(inputs["q_hidden"], dtype=np.float32)
    pd = np.asarray(inputs["pd_hidden"], dtype=np.float32)
    nd = np.asarray(inputs["nd_hidden"], dtype=np.float32)
    W = np.asarray(inputs["W"], dtype=np.float32)
    b = np.asarray(inputs["b"], dtype=np.float32)

    w_t = np.ascontiguousarray(
        _fp8(SW * W).reshape(KC, 2, 128, D).transpose(2, 0, 1, 3)
    ).reshape(128, KC * 2 * D)
    b2 = np.ascontiguousarray(
        np.stack([SW * b, 2.0 * b], axis=1).astype(np.float32)
    )
    mp = np.asarray(inputs["pd_mask"], dtype=np.float32) * (2.0 ** -8)
    mn = np.asarray(inputs["nd_mask"], dtype=np.float32) * (2.0 ** -8)

    nsel = np.zeros((128, 2, 2, 16), dtype=FP8_NP)
    for p in range(2):
        nsel[:, p, 0, 2 * p] = 1.0
        nsel[:, p, 1, 2 * p + 1] = 1.0
    nsel = nsel.reshape(128, 64)
    blk4 = np.zeros((4, 128), dtype=BF16_NP)
    for j in range(4):
        blk4[j, 32 * j : 32 * (j + 1)] = 1
    e4 = np.zeros((128, 4), dtype=BF16_NP)
    for gg in range(4):
        e4[32 * gg : 32 * (gg + 1), gg] = 1

    maps = []
    for cix in range(NCORES):
        sl = slice(cix * BC, (cix + 1) * BC)
        # query: tokens b-major as one 512-col tile
        qc = q[sl].reshape(BC * LQ, KC, 2, 128).transpose(3, 1, 2, 0)
        # masks -> [j(4), g(8)=2u+side, 512] bf16
        mall = np.zeros((4, 8, LD), dtype=np.float32)
        for u in range(NG):
            for j in range(4):
                mall[j, 2 * u + 0] = mp[sl][4 * u + j]
                mall[j, 2 * u + 1] = mn[sl][4 * u + j]
        maps.append(
            {
                "qt": np.ascontiguousarray(_fp8(qc)).reshape(128, KC * 2 * LD),
                "pdq": _pack_docs(pd[sl]),
                "ndq": _pack_docs(nd[sl]),
                "W": w_t,
                "b2": b2,
                "mall": mall.astype(BF16_NP).reshape(4, 8 * LD),
                "nsel": nsel,
                "blk4": blk4,
                "e4": e4,
            }
        )
    return maps


def run(inputs, **kw):
    """Run on 8 cores; returns (out [128,2] fp32, BassKernelResults)."""
    nc = _get_nc()
    res = run_bass_kernel_spmd(nc, _in_maps(inputs), list(range(NCORES)), **kw)
    out = np.concatenate(
        [np.asarray(res.results[c]["out"], dtype=np.float32) for c in range(NCORES)],
        axis=0,
    )
    return out, res


def kernel(**inputs) -> np.ndarray:
    out, _ = run(inputs)
    return out
